# revision 3
# baseline (speedup 1.0000x reference)
"""Trainium2 Bass kernel for nn_InformPooling (segment mean-pooling).

Strategy (pure data parallel, batch b -> core b):
  out[n] = mean over rows [s_n, e_n) of v, for three value tensors at ratios
  (1.0, 0.5, 0.25), channels concatenated.

  Decompose the exclusive prefix sum at a boundary position p as
      bcsum(p) = csumP[p // 128] + within_tile_prefix(p)
  where csumP is the exclusive cumsum of per-tile (128-row) sums.

  Device work per core:
    - per 128-row tile j: one PE matmul  masks_j.T @ v_j  -> within-tile
      prefixes for every boundary that falls in tile j (mask columns are
      host-built 0/1 prefix masks, CAP slots per tile)
    - per tile: PE matmuls v_j.T @ ones -> per-tile sums as PSUM columns
    - DVE scan over tile sums -> exclusive csumP; PE transpose -> [ntiles, C]
    - boundary rows + csumP written to scratch DRAM; indirect-DMA row
      gathers in n-order; combine + scale by 1/cnt on DVE; write output.

  Host work: only index math on start/duration ([8,512] -> s/e/cnt) and
  building the small mask/table inputs.
"""

import sys
from contextlib import ExitStack

import numpy as np

sys.path.insert(0, "/opt/trn_rl_repo")

import concourse.bass as bass
import concourse.bacc as bacc
import concourse.tile as tile
from concourse import mybir
from concourse.bass_utils import run_bass_kernel_spmd
from concourse.masks import make_identity

F32 = mybir.dt.float32
I32 = mybir.dt.int32
ADD = mybir.AluOpType.add
SUB = mybir.AluOpType.subtract

RATIOS = (1.0, 0.5, 0.25)
TS = (16384, 8192, 4096)
CS = (128, 128, 256)
COFF = (0, 128, 256)
CHS = (16, 16, 8)  # value tiles per DMA chunk
NSEG = 512
NB = 8
EPS = 1e-3


def _host_prep(start, duration):
    """Index math + mask/table construction. start/duration: [NB, NSEG] f32."""
    start = np.asarray(start, np.float32)
    duration = np.asarray(duration, np.float32)
    per_r = []
    for r in range(3):
        T = TS[r]
        nt = T // 128
        rr = np.float32(RATIOS[r])
        s = np.minimum(np.floor(start * rr).astype(np.int32), T - 1)
        e = np.minimum(
            np.ceil((start + duration + np.float32(EPS)) * rr).astype(np.int32), T - 1
        )
        cnt = e - s
        invc = np.where(
            cnt > 0, np.float32(1.0) / np.maximum(cnt, 1).astype(np.float32), 0.0
        ).astype(np.float32)

        p = np.concatenate([s, e], axis=1)  # [NB, 2N]
        tj = p // 128
        off = p % 128
        ranks = np.zeros_like(p)
        cols = np.zeros_like(p)
        cap = 0
        for b in range(NB):
            order = np.argsort(tj[b], kind="stable")
            tjs = tj[b][order]
            cc = np.arange(2 * NSEG) - np.searchsorted(tjs, tjs)
            cap = max(cap, int(cc.max()) + 1)
            ranks[b][order] = cc * nt + tjs  # BND row = slot*ntiles + tile
            cols[b][order] = tjs * cap + cc  # mask col = tile*CAP + slot
        CAP = cap
        rks = nt * CAP
        oo = np.zeros((NB, rks), np.int32)
        for b in range(NB):
            # cols were computed with the per-core running cap; recompute with
            # the final CAP so all cores share one layout
            order = np.argsort(tj[b], kind="stable")
            tjs = tj[b][order]
            cc = np.arange(2 * NSEG) - np.searchsorted(tjs, tjs)
            cols[b][order] = tjs * CAP + cc
            oo[b][cols[b]] = off[b]
        mk = (np.arange(128)[None, :, None] < oo[:, None, :]).astype(np.float32)
        tbl = np.stack(
            [tj[:, :NSEG], ranks[:, :NSEG], tj[:, NSEG:], ranks[:, NSEG:]], axis=1
        ).astype(np.int32)  # [NB, 4, 512]
        per_r.append(
            dict(CAP=CAP, mk=mk, tbl=tbl.reshape(NB, 4, 4, 128),
                 invc=invc.reshape(NB, 4, 128))
        )
    return per_r


def _value_phase(ctx, tc, nc, r, CAP, val, mk, tbl, invc, outp, bnd, cp,
                 ones_sb, zeros_sb, ident):
    T, C, CH = TS[r], CS[r], CHS[r]
    nt = T // 128
    rks = nt * CAP
    nch = nt // CH
    nhalf = C // 128

    mpool = ctx.enter_context(tc.tile_pool(name=f"mk{r}", bufs=1))
    vpool = ctx.enter_context(tc.tile_pool(name=f"v{r}", bufs=3))
    spool = ctx.enter_context(tc.tile_pool(name=f"st{r}", bufs=1))
    small = ctx.enter_context(tc.tile_pool(name=f"sm{r}", bufs=1))
    gpool = ctx.enter_context(tc.tile_pool(name=f"g{r}", bufs=2))
    bps = ctx.enter_context(tc.tile_pool(name=f"bps{r}", bufs=2, space="PSUM"))
    pps = ctx.enter_context(tc.tile_pool(name=f"pps{r}", bufs=1, space="PSUM"))

    mk_sb = mpool.tile([128, rks], F32)
    nc.sync.dma_start(mk_sb[:], mk.ap())
    stg = spool.tile([CAP, nt, C], F32)
    p_ps = [pps.tile([128, nt], F32, tag=f"p{h}", name=f"p_ps{r}_{h}")
            for h in range(nhalf)]

    val_t = val.ap().rearrange("(n p) c -> p n c", p=128)
    for ch in range(nch):
        v_sb = vpool.tile([128, CH, C], F32)
        nc.sync.dma_start(v_sb[:], val_t[:, ch * CH:(ch + 1) * CH, :])
        for jj in range(CH):
            j = ch * CH + jj
            ps = bps.tile([CAP, C], F32, tag="bnd")
            nc.tensor.matmul(
                ps[:], mk_sb[:, j * CAP:(j + 1) * CAP], v_sb[:, jj, :],
                start=True, stop=True,
            )
            if j % 2 == 0:
                nc.vector.tensor_copy(stg[:, j, :], ps[:])
            else:
                nc.scalar.copy(stg[:, j, :], ps[:])
            for h in range(nhalf):
                nc.tensor.matmul(
                    p_ps[h][:, j:j + 1],
                    v_sb[:, jj, h * 128:(h + 1) * 128], ones_sb[:],
                    start=True, stop=True,
                )
    nc.sync.dma_start(bnd.ap(), stg[:])

    # tile sums -> exclusive csumP -> [ntiles, C] in DRAM
    cp_sb = small.tile([nt, C], F32)
    for h in range(nhalf):
        p_sb = small.tile([128, nt], F32, tag=f"psb{h}")
        nc.vector.tensor_copy(p_sb[:], p_ps[h][:])
        cs = small.tile([128, nt], F32, tag=f"cs{h}")
        nc.vector.memset(cs[:, 0:1], 0.0)
        nc.vector.tensor_tensor_scan(
            cs[:, 1:nt], p_sb[:, 0:nt - 1], zeros_sb[:, 0:nt - 1],
            0.0, ADD, ADD,
        )
        tp = bps.tile([nt, 128], F32, tag="tp")
        nc.tensor.transpose(tp[:], cs[:], ident[:])
        nc.vector.tensor_copy(cp_sb[:, h * 128:(h + 1) * 128], tp[:])
    nc.sync.dma_start(cp.ap(), cp_sb[:])

    tbl_sb = small.tile([128, 4, 4], I32)
    nc.sync.dma_start(tbl_sb[:], tbl.ap().rearrange("a k p -> p a k"))
    iv_sb = small.tile([128, 4], F32)
    nc.sync.dma_start(iv_sb[:], invc.ap().rearrange("k p -> p k"))

    bnd_flat = bnd.ap().rearrange("i j c -> (i j) c")
    for k in range(4):
        g = {}
        for name, a in (("as_", 0), ("bs", 1), ("ae", 2), ("be", 3)):
            gt = gpool.tile([128, C], F32, tag=f"g{name}")
            src = cp.ap() if a in (0, 2) else bnd_flat
            nc.gpsimd.indirect_dma_start(
                out=gt[:], out_offset=None, in_=src,
                in_offset=bass.IndirectOffsetOnAxis(ap=tbl_sb[:, a, k:k + 1], axis=0),
            )
            g[name] = gt
        t1 = gpool.tile([128, C], F32, tag="t1")
        nc.vector.tensor_tensor(t1[:], g["ae"][:], g["be"][:], ADD)
        t2 = gpool.tile([128, C], F32, tag="t2")
        nc.vector.tensor_tensor(t2[:], g["as_"][:], g["bs"][:], ADD)
        t3 = gpool.tile([128, C], F32, tag="t3")
        nc.vector.tensor_tensor(t3[:], t1[:], t2[:], SUB)
        o_sb = gpool.tile([128, C], F32, tag="o")
        nc.vector.tensor_scalar_mul(o_sb[:], t3[:], iv_sb[:, k:k + 1])
        nc.sync.dma_start(
            outp.ap()[k * 128:(k + 1) * 128, COFF[r]:COFF[r] + C], o_sb[:]
        )


def _build_program(caps):
    nc = bacc.Bacc("TRN2", target_bir_lowering=False, debug=False, num_devices=NB)
    vals = [nc.dram_tensor(f"val{r}", [TS[r], CS[r]], F32, kind="ExternalInput")
            for r in range(3)]
    mks = [nc.dram_tensor(f"mk{r}", [128, (TS[r] // 128) * caps[r]], F32,
                          kind="ExternalInput") for r in range(3)]
    tbls = [nc.dram_tensor(f"tbl{r}", [4, 4, 128], I32, kind="ExternalInput")
            for r in range(3)]
    invcs = [nc.dram_tensor(f"invc{r}", [4, 128], F32, kind="ExternalInput")
             for r in range(3)]
    outp = nc.dram_tensor("out", [NSEG, 512], F32, kind="ExternalOutput")
    bnds = [nc.dram_tensor(f"bnd{r}", [caps[r], TS[r] // 128, CS[r]], F32)
            for r in range(3)]
    cps = [nc.dram_tensor(f"cp{r}", [TS[r] // 128, CS[r]], F32) for r in range(3)]

    with tile.TileContext(nc) as tc:
        with ExitStack() as gctx:
            glob = gctx.enter_context(tc.tile_pool(name="glob", bufs=1))
            ones_sb = glob.tile([128, 1], F32)
            nc.vector.memset(ones_sb[:], 1.0)
            zeros_sb = glob.tile([128, 128], F32)
            nc.vector.memset(zeros_sb[:], 0.0)
            ident = glob.tile([128, 128], F32)
            make_identity(nc, ident[:])
            for r in range(3):
                with ExitStack() as ctx:
                    _value_phase(ctx, tc, nc, r, caps[r], vals[r], mks[r],
                                 tbls[r], invcs[r], outp, bnds[r], cps[r],
                                 ones_sb, zeros_sb, ident)
    nc.compile()
    return nc


_CACHE = {}


def kernel(value0, value1, value2, start, duration):
    vals = [np.asarray(v, np.float32) for v in (value0, value1, value2)]
    per_r = _host_prep(start, duration)
    caps = tuple(d["CAP"] for d in per_r)
    if caps not in _CACHE:
        _CACHE[caps] = _build_program(caps)
    nc = _CACHE[caps]

    in_maps = []
    for b in range(NB):
        m = {}
        for r in range(3):
            m[f"val{r}"] = np.ascontiguousarray(vals[r][b])
            m[f"mk{r}"] = np.ascontiguousarray(per_r[r]["mk"][b])
            m[f"tbl{r}"] = np.ascontiguousarray(per_r[r]["tbl"][b])
            m[f"invc{r}"] = np.ascontiguousarray(per_r[r]["invc"][b])
        in_maps.append(m)

    res = run_bass_kernel_spmd(nc, in_maps, list(range(NB)))
    return np.stack([res.results[b]["out"] for b in range(NB)], axis=0)
